# revision 43
# baseline (speedup 1.0000x reference)
"""BCM_Conv2d_fft kernel for Trainium2 (8 NeuronCores, batch-parallel).

The reference is a block-circulant 3x3 conv computed via per-block
rfft/irfft over the channel-block axis (block size 8). Per-frequency the
block products are independent, so in a real-DFT channel basis the
256->256 channel mixing matrix of each conv tap is block-diagonal with
frequency groups {f0:32, f4:32, f1:64, f2:64, f3:64}. Grouping
{f0,f4,f1} -> chunk0 and {f2,f3} -> chunk1 makes every tap's mixing
matrix chunk-diagonal: the conv needs 9 matmuls per output tile per
chunk instead of 18 - half the direct-conv PE work.

Device pipeline per core (one image):
  1. fwd:  xhat = A @ x       per pixel (A = real-DFT, freq-major rows)
  2. conv: ohat = sum_pos M_pos @ shift(xhat)   (chunk-diagonal M)
  3. inv:  out  = Ainv @ ohat + b   (bias on DVE for chunk0, ACT for 1)

All matmul operands are fp16 (PE streams 16-bit moving operands at
1 col/cycle vs 1/2 for fp32; PSUM accumulation stays fp32), inputs and
outputs ship as fp16 and are converted on the host. Start-up is DMA-
latency-bound, so: dummy matmuls on a memset scratch region warm the PE
HAM clock gate, a tiny lead transfer on each queue absorbs the DGE ring
startup, the first x rows + a small bootstrap copy of the fwd weights
are the only early transfers in flight, and the weight bulk rides one
big-descriptor DMA. The final output tile is processed as two half
tiles so the drain after the last matmul is short.
"""

import os

import numpy as np

import concourse.bacc as bacc
import concourse.mybir as mybir
import concourse.tile as tile
from concourse.bass import ts
from concourse.bass_utils import run_bass_kernel_spmd

N_CORES = 8
C = 256
H = W = 56
HP = H + 2
KK = 3
BS = 8
L = H * W
RPT = 8                  # fwd rows per tile
NT = RPT * W             # 448 pixels per full output tile
MCH = C // 128           # 2 channel chunks

F32 = mybir.dt.float32
F16 = mybir.dt.float16

# weight block column indices in the packed wts tensor [128, 26*128]
# layout: [fwd c0 (2) | fwd c1 (2) | conv chunk0 (9) | conv chunk1 (9) |
# inv(4)] so each stage/chunk group is contiguous; the fwd group also
# ships early as a separate small bootstrap copy.
FWD_BLK = lambda i, c: c * MCH + i            # i = in chunk, c = out chunk
CONV_BLK = lambda pos, c: 4 + c * 9 + pos
INV_BLK = lambda k, m: 22 + k * MCH + m
N_BLKS = 26

# Dummy matmuls warm the PE HAM clock gate (4096-cycle activity window;
# any >~1us idle inside it postpones the 1.2->2.4GHz unthrottle) while
# the first input DMAs complete - the PE would sit idle regardless, and
# ~9 cold dummies flip the gate so everything real runs at 2.4GHz.
N_WARM = 12

# output jobs: (start pixel, npix); all row-aligned. Six full 8-row
# tiles, then a 7-row and a 1-row job so the tail (last matmul -> last
# byte shipped) is shallow.
JOBS = [(nt * NT, NT) for nt in range(6)] + [(6 * NT, 7 * W),
                                             (6 * NT + 7 * W, W)]

LAST_RESULT = None


def _freq_matrices(w: np.ndarray):
    """Build A [256,256], Ms (9x [256,256] chunk-diag), Ainv from w."""
    F = np.zeros((8, 8))
    FI = np.fft.rfft(np.eye(8), axis=-1)
    F[0] = FI[:, 0].real
    F[1], F[2] = FI[:, 1].real, FI[:, 1].imag
    F[3], F[4] = FI[:, 2].real, FI[:, 2].imag
    F[5], F[6] = FI[:, 3].real, FI[:, 3].imag
    F[7] = FI[:, 4].real

    def fm(bk, comp):
        if comp == 0:
            return bk
        if comp == 7:
            return 32 + bk
        if comp in (1, 2):
            return 64 + 2 * bk + (comp - 1)
        if comp in (3, 4):
            return 128 + 2 * bk + (comp - 3)
        return 192 + 2 * bk + (comp - 5)

    A = np.zeros((256, 256))
    for bk in range(32):
        for comp in range(8):
            A[fm(bk, comp), bk * 8:(bk + 1) * 8] = F[comp]
    Ainv = np.linalg.inv(A)

    wf = np.fft.rfft(w.astype(np.float64), axis=-1)  # [32, 288, 5]
    Ms = []
    for pos in range(9):
        M = np.zeros((256, 256))
        for pb in range(32):
            for kb in range(32):
                kc = pos * 32 + kb
                M[fm(pb, 0), fm(kb, 0)] += wf[pb, kc, 0].real
                M[fm(pb, 7), fm(kb, 7)] += wf[pb, kc, 4].real
                for fi in range(3):
                    re_i, im_i = 1 + 2 * fi, 2 + 2 * fi
                    Wr, Wi = wf[pb, kc, fi + 1].real, wf[pb, kc, fi + 1].imag
                    M[fm(pb, re_i), fm(kb, re_i)] += Wr
                    M[fm(pb, re_i), fm(kb, im_i)] += -Wi
                    M[fm(pb, im_i), fm(kb, re_i)] += Wi
                    M[fm(pb, im_i), fm(kb, im_i)] += Wr
        Ms.append(M)
    return A, Ms, Ainv


def _pack_weights(w: np.ndarray, b: np.ndarray) -> np.ndarray:
    """-> [128, 26*128+2] fp16: lhsT blocks (fwd | bias cols | conv, inv)."""
    A, Ms, Ainv = _freq_matrices(w)
    wts = np.zeros((128, N_BLKS * 128 + MCH), np.float16)
    wts[:, 4 * 128:4 * 128 + MCH] = b.reshape(MCH, 128).T

    def put(idx, mat):  # mat [K=128, M=128] already transposed for lhsT
        off = 0 if idx < 4 else MCH
        wts[:, off + idx * 128:off + (idx + 1) * 128] = mat.astype(np.float16)

    sl = lambda i: slice(i * 128, (i + 1) * 128)
    for i in range(MCH):
        for c in range(MCH):
            put(FWD_BLK(i, c), A[sl(c), sl(i)].T)
    for pos in range(9):
        for c in range(MCH):
            put(CONV_BLK(pos, c), Ms[pos][sl(c), sl(c)].T)
    for k in range(MCH):
        for m in range(MCH):
            put(INV_BLK(k, m), Ainv[sl(m), sl(k)].T)
    return wts


def _kernel_body(tc, x, wts, out):
    nc = tc.nc
    with (
        tc.tile_pool(name="const", bufs=1) as const_pool,
        tc.tile_pool(name="xp", bufs=1) as xp_pool,
        tc.tile_pool(name="xh", bufs=1) as xh_pool,
        tc.tile_pool(name="oh", bufs=6) as oh_pool,
        tc.tile_pool(name="ob", bufs=4) as ob_pool,
        tc.tile_pool(name="psf", bufs=3, space="PSUM") as psf_pool,
        tc.tile_pool(name="psc", bufs=3, space="PSUM") as psc_pool,
        tc.tile_pool(name="psi", bufs=2, space="PSUM") as psi_pool,
    ):
        # scratch for PE warm-up matmuls: zeroed first thing on DVE
        # (keeps all three DMA-trigger engines free for transfers)
        dum = const_pool.tile([128, NT], F16)
        nc.vector.memset(dum[:], 0.0)

        # boot: fwd weights + 2 fp16 bias columns (one early transfer)
        boot = const_pool.tile([128, 4 * 128 + MCH], F16)
        bulk = const_pool.tile([128, 22 * 128], F16)  # conv + inv weights

        def blk(idx):
            if idx < 4:
                return boot[:, ts(idx, 128)]
            return bulk[:, ts(idx - 4, 128)]

        bias_sb = const_pool.tile([128, MCH], F32)
        xq = []
        for i in range(MCH):
            xq_t = xp_pool.tile([128, HP * HP], F16, tag=f"xp{i}")
            xq.append(xq_t)

        # DMA plan. Aggregate HBM bandwidth (~300 GB/s) is shared by the
        # three issuing queues, each dma_start pays ~1-2us of fixed
        # ring-start + completion-receipt latency, and throughput rises
        # with descriptor (per-partition line) size. So: the early
        # in-flight set is exactly {first x rows of both chunks, the
        # 128KB fwd bootstrap}, in few medium transfers; the conv/inv
        # weights follow as separate 2304B-line transfers (separate so
        # conv chunk0 doesn't wait on the whole bulk's completion
        # semaphore). sync: x chunk0; gpsimd: x chunk1; scalar (stream
        # starts with the ACT table load): weights.
        # sync (HW queue): x chunk0; gpsimd (SWDGE): x chunk1; scalar
        # (HW queue, no ACT table load since no activation is used):
        # fwd bootstrap + conv/inv weights. Fine-grained early x splits
        # so completion receipts (~1-2us each) pipeline.
        row_splits = [0, 8, 16, 24, 40, HP]
        eng = [nc.sync, nc.gpsimd]
        for r0, r1 in zip(row_splits[:-1], row_splits[1:]):
            for i in range(MCH):
                eng[i].dma_start(
                    out=xq[i][:, r0 * HP:r1 * HP],
                    in_=x[ts(i, 128), r0:r1, :].rearrange("p h w -> p (h w)"),
                )
        nc.scalar.dma_start(out=boot[:], in_=wts[:, 0:4 * 128 + MCH])
        # conv chunk0's first three position blocks ship separately so
        # conv job 0 can start accumulating as soon as the fwd lead is
        # done instead of waiting on the whole chunk's completion sem
        nc.scalar.dma_start(out=bulk[:, 0:3 * 128],
                            in_=wts[:, 4 * 128 + MCH:7 * 128 + MCH])
        nc.scalar.dma_start(out=bulk[:, 3 * 128:9 * 128],
                            in_=wts[:, 7 * 128 + MCH:13 * 128 + MCH])
        # same early-positions split for conv chunk1
        nc.scalar.dma_start(out=bulk[:, 9 * 128:12 * 128],
                            in_=wts[:, 13 * 128 + MCH:16 * 128 + MCH])
        nc.scalar.dma_start(out=bulk[:, 12 * 128:18 * 128],
                            in_=wts[:, 16 * 128 + MCH:22 * 128 + MCH])
        nc.scalar.dma_start(out=bulk[:, 18 * 128:],
                            in_=wts[:, 22 * 128 + MCH:])

        # dummy matmuls on the scratch region warm the PE clock gate
        # while the first inputs stream in; more are inserted between
        # early fwd tiles (emit_warm) to bridge DMA-wait gaps - any PE
        # idle >~1us inside the HAM activity window postpones the
        # 1.2->2.4 GHz unthrottle by several microseconds.
        def emit_warm(n):
            for _ in range(n):
                ps_w = psc_pool.tile([128, NT], F32, tag="psc")
                nc.tensor.matmul(ps_w[:], lhsT=dum[:, 0:128],
                                 rhs=dum[:, 0:NT], start=True, stop=True)

        emit_warm(N_WARM)

        # widen the fp16 bias columns to f32 once (tensor_scalar needs
        # a float32 scalar operand)
        nc.vector.tensor_copy(bias_sb[:], boot[:, 4 * 128:])

        # xhat: frequency-basis transform of the whole padded image (the
        # borders of x are zero, so xhat borders transform to zero too).
        xhat = []
        for c in range(MCH):
            xh_t = xh_pool.tile([128, HP * HP], F16, tag=f"xh{c}")
            xhat.append(xh_t)
        # padded-row ranges per fwd tile: small leading tiles so the
        # first matmuls only need a few x rows resident, 8-row steady
        # tiles after
        fwd_rows = [(0, 4), (4, 8), (8, 12), (12, 16), (16, 24),
                    (24, 32), (32, 40), (40, 48), (48, 56), (56, HP)]

        def fwd_tile(it):
            """Transform padded pixel rows [r0, r1) of the image."""
            r0, r1 = fwd_rows[it]
            npx = (r1 - r0) * HP
            for c in range(MCH):
                ps = psf_pool.tile([128, RPT * HP], F32, tag="psf")
                for i in range(MCH):
                    rhs = xq[i][:, r0 * HP: r1 * HP]
                    nc.tensor.matmul(
                        ps[:, :npx], lhsT=blk(FWD_BLK(i, c)), rhs=rhs,
                        start=(i == 0), stop=(i == MCH - 1),
                    )
                nc.vector.tensor_copy(
                    xhat[c][:, r0 * HP: r1 * HP], ps[:, :npx]
                )

        def conv_job(p0, npx):
            """Freq-domain conv for out pixels [p0, p0+npx) -> ohat."""
            r0 = p0 // W
            nr = npx // W
            ohat = []
            for c in range(MCH):
                ps = psc_pool.tile([128, NT], F32, tag="psc")
                n_mm = 0
                for kh in range(KK):
                    for kw in range(KK):
                        pos = kh * KK + kw
                        xhv = xhat[c][:].rearrange("p (h w) -> p h w", h=HP)
                        rhs = xhv[:, r0 + kh: r0 + kh + nr, kw: kw + W]
                        nc.tensor.matmul(
                            ps[:, :npx], lhsT=blk(CONV_BLK(pos, c)), rhs=rhs,
                            start=(n_mm == 0), stop=(n_mm == KK * KK - 1),
                        )
                        n_mm += 1
                oh = oh_pool.tile([128, NT], F16, tag="oh")
                nc.vector.tensor_copy(oh[:, :npx], ps[:, :npx])
                ohat.append(oh)
            return ohat

        # output ships rotate across the three queues so no queue backs
        # up and the two tail jobs land on the idle HW queues
        out_eng = [nc.scalar, nc.sync, nc.gpsimd]

        def inv_job(p0, npx, ohat, ob, off):
            """Inverse transform + bias for out pixels [p0, p0+npx)."""
            for m in range(MCH):
                ps = psi_pool.tile([128, NT], F32, tag="psi")
                for k in range(MCH):
                    nc.tensor.matmul(
                        ps[:, :npx], lhsT=blk(INV_BLK(k, m)),
                        rhs=ohat[k][:, :npx],
                        start=(k == 0), stop=(k == MCH - 1),
                    )
                # bias-add on DVE (avoiding nc.scalar.activation keeps
                # the ACT table load out of the preamble, so scalar's
                # weight DMA triggers issue ~1.3us earlier)
                nc.vector.tensor_scalar_add(
                    ob[:, off:off + npx, m], ps[:, :npx],
                    bias_sb[:, m:m + 1]
                )

        # Interleave: fwd runs 3 tiles ahead of conv (conv of out row r
        # reads padded xhat rows [r, r+10) = fwd tiles r/8 and r/8+1),
        # and each job's inv is emitted after the NEXT fwd tile so the
        # ohat PSUM->SBUF copies hide under fwd matmuls.
        fwd_state = [0]

        def emit_fwd_until(rows_needed):
            while (fwd_state[0] < len(fwd_rows)
                   and fwd_rows[fwd_state[0]][0] < rows_needed):
                fwd_tile(fwd_state[0])
                fwd_state[0] += 1

        emit_fwd_until(8)    # tiles (0,4),(4,8)
        emit_warm(2)         # bridge the x rows [8,16) DMA wait
        emit_fwd_until(16)   # tiles (8,12),(12,16)
        emit_warm(3)         # bridge the x rows [16,24) / weight seam
        emit_fwd_until(24)
        # mid jobs ship in pairs (one 896-px interleaved transfer: big
        # lines, one trigger); the two tail jobs ship individually so
        # the final drain is shallow
        pair_ob = None
        pair_p0 = pair_off = 0
        for ji, (p0, npx) in enumerate(JOBS):
            emit_fwd_until(p0 // W + 10)
            ohat = conv_job(p0, npx)
            emit_fwd_until(p0 // W + 26)
            if ji < 6:
                if ji % 2 == 0:
                    pair_ob = ob_pool.tile([128, 2 * NT, MCH], F16,
                                           tag="ob")
                    pair_p0, pair_off = p0, 0
                inv_job(p0, npx, ohat, pair_ob, pair_off)
                pair_off += npx
                if ji % 2 == 1:
                    out_eng[(ji // 2) % 3].dma_start(
                        out=out[:, pair_p0:pair_p0 + pair_off, :],
                        in_=pair_ob[:, :pair_off, :],
                    )
            else:
                ob = ob_pool.tile([128, NT, MCH], F16, tag="obt")
                inv_job(p0, npx, ohat, ob, 0)
                dma_eng = nc.scalar if ji == 6 else nc.sync
                dma_eng.dma_start(
                    out=out[:, p0:p0 + npx, :], in_=ob[:, :npx, :]
                )


def _build_nc():
    nc = bacc.Bacc("TRN2", target_bir_lowering=False, debug=False)
    x = nc.dram_tensor("x", [C, HP, HP], F16, kind="ExternalInput").ap()
    wts = nc.dram_tensor("wts", [128, N_BLKS * 128 + MCH], F16,
                         kind="ExternalInput").ap()
    # pixel-major interleaved output: out[p, pix, m] = channel m*128+p,
    # un-interleaved on the host; doubles the DMA line size per ship
    out = nc.dram_tensor("out", [128, L, MCH], F16, kind="ExternalOutput").ap()
    with tile.TileContext(nc) as tc:
        _kernel_body(tc, x, wts, out)
    nc.compile()
    return nc


def kernel(x: np.ndarray, w: np.ndarray, b: np.ndarray) -> np.ndarray:
    global LAST_RESULT
    xp = np.pad(np.asarray(x, np.float32), ((0, 0), (0, 0), (1, 1), (1, 1)))
    xp = np.ascontiguousarray(xp.astype(np.float16))
    wts = _pack_weights(np.asarray(w, np.float32), np.asarray(b, np.float32))

    nc = _build_nc()
    in_maps = [{"x": xp[i], "wts": wts} for i in range(N_CORES)]
    trace = bool(int(os.environ.get("KERNEL_PROFILE", "0")))
    res = None
    last_err = None
    for attempt in range(3):
        try:
            res = run_bass_kernel_spmd(
                nc,
                in_maps,
                core_ids=list(range(N_CORES)),
                trace=trace,
            )
            break
        except Exception as e:  # transient device wedge -> retry
            last_err = e
    if res is None:
        raise last_err
    LAST_RESULT = res
    outs = []
    for i in range(N_CORES):
        o = res.results[i]["out"].astype(np.float32)   # [128, L, MCH]
        outs.append(o.transpose(2, 0, 1).reshape(C, H, W))
    return np.stack(outs, axis=0)
